# revision 15
# baseline (speedup 1.0000x reference)
"""Trainium2 Bass kernel for nn_CrossAttention (B=8, Sq=Skv=2048, D=1024, C=768).

Strategy: data-parallel over batch — each of the 8 NeuronCores computes one
batch element's full cross-attention.

The projection chain is reassociated so every big contraction runs against
the NARROW context dim (C=768) instead of D=1024, and the K/V projections
disappear entirely (all exact identities, weights folded on host):

  scores = (x @ M + bqk) @ ctx^T          M   = Wq^T @ Wk   [D, C]
                                          bqk = bq @ Wk     [C]
  (bk drops: its score term is constant over k -> cancels in softmax)
  att    = softmax(scores / sqrt(D))
  final  = (e @ ctx)/sums @ WVO + bo''    WVO = (Wo @ Wv)^T [C, D]
                                          bo''= bo + Wo @ bv

Precision plan (2e-2 gate; this config emulates to ~1.67e-2):
  - scores matmul: fp8e4m3 DoubleRow (xm8 @ ctx8), 1/sqrt(D) folded into Exp.
  - att @ ctx matmul: fp8 DoubleRow with CENTERED attention weights:
      e_hat = EC + e8/ESC,  e8 = fp8((e - EC) * ESC)
    e clusters tightly around its mean (~1.1, std 0.52), so quantizing the
    centered residual cuts the fp8 noise ~3x. The rank-1 correction
    EC * colsum(ctxk8) is folded into the PSUM-evac bias (host-computed,
    exact, free). Softmax sums use the bf16 e (DVE pair-add tree), so the
    denominator matches the pre-quantization weights.
  - phase-1 projection and the final @WVO stay bf16: fp8 there busts the
    error budget (emulated 2.1e-2 / 2.5e-2).
  - output written bf16 (halves the tail DMA; +3e-4 error).

Per-core phases:
  phase 1 (q chunks of 512): xm^T[c,q] = M^T x^T + bqk -> fp8 resident.
    Chunk 0 runs it-OUTER (each arriving (M[it], x0[it]) DMA pair
    immediately feeds 6 matmuls across all cs psum banks) so the PE starts
    ~2us after the first tiles land instead of waiting for the full 2.5MB.
  phase 2 (per q block): scores^T fp8-DR -> Exp (bf16 e) -> centered fp8 e8;
    DVE accumulates softmax sums from bf16 e; ones-matmul + e0-trick
    transpose -> 1/sums. outp^T[c,q] = ctxk8^T @ e8 (fp8 DR over kt pairs),
    evac fuses *1/ESC + EC*colsum bias. final[q,o] = outp^T.T @ WVO (bf16);
    evac fuses (*recip + bo'') in one DVE op -> bf16 -> DMA out.

DMA: 3 queue-engines (sync/vector/gpsimd) carry the phase-1-critical M/x
stream in need order; scalar's queue carries ctx8/ctxk8/wvo/consts in the
background. Full-width per-it pieces (fewer, larger descriptors).
"""

import numpy as np
import ml_dtypes

import concourse.bass as bass  # noqa: F401
import concourse.mybir as mybir
import concourse.tile as tile
from concourse import bacc
from concourse.bass_utils import run_bass_kernel_spmd

# ---- problem shapes (hardcoded) ----
B, SQ, SKV, D, C = 8, 2048, 2048, 1024, 768
P = 128
DT = D // P          # 8  d-tiles
CT = C // P          # 6  c-tiles
KT = SKV // P        # 16 k-tiles
QB = 512             # q block width
NQB = SQ // QB       # 4 q blocks
SCALE = 1.0 / np.sqrt(np.float32(D))

EC = 1.105           # centering constant for e = exp(score)
ESC = 8.0            # fp8 scale for the centered residual

F32 = mybir.dt.float32
BF16 = mybir.dt.bfloat16
FP8 = mybir.dt.float8e4
AF = mybir.ActivationFunctionType
ALU = mybir.AluOpType
DR = mybir.MatmulPerfMode.DoubleRow

_NC_CACHE = {}


def build():
    if "nc" in _NC_CACHE:
        return _NC_CACHE["nc"]
    nc = bacc.Bacc(trn_type="TRN2", num_swdge_queues=4)

    # ---- DRAM I/O (per-core slices; names = in_map keys) ----
    xT = nc.dram_tensor("xT", [D, SQ], BF16, kind="ExternalInput")
    ctx8T = nc.dram_tensor("ctx8T", [C, SKV], FP8, kind="ExternalInput")
    ctxk8 = nc.dram_tensor("ctxk8", [SKV, C], FP8, kind="ExternalInput")
    Mh = nc.dram_tensor("Mh", [D, C], BF16, kind="ExternalInput")
    wvoh = nc.dram_tensor("wvoh", [C, D], BF16, kind="ExternalInput")
    bqkh = nc.dram_tensor("bqkh", [P, CT], F32, kind="ExternalInput")
    bob = nc.dram_tensor("bob", [P, D], F32, kind="ExternalInput")
    csumb = nc.dram_tensor("csumb", [P, CT], F32, kind="ExternalInput")
    onesmat = nc.dram_tensor("onesmat", [P, P], BF16, kind="ExternalInput")
    e0two = nc.dram_tensor("e0two", [P, 2], BF16, kind="ExternalInput")
    ecbh = nc.dram_tensor("ecbh", [P, 1], F32, kind="ExternalInput")
    out = nc.dram_tensor("out", [SQ, D], BF16, kind="ExternalOutput")

    with tile.TileContext(nc) as tc:
        with tc.tile_pool(name="persist", bufs=1) as persist:
            ctx8_sb = persist.tile([P, CT, SKV], FP8, name="ctx8_sb")
            ctxk_sb = persist.tile([P, KT, C], FP8, name="ctxk_sb")
            xm_tiles = [persist.tile([P, CT, QB], FP8,
                                     name=f"xm_sb{qc}")
                        for qc in range(NQB)]
            m_tiles = [persist.tile([P, C], BF16, name=f"m_sb{it}")
                       for it in range(DT)]
            xt_tiles = [[persist.tile([P, QB], BF16, name=f"xt{qc}_{it}")
                         for it in range(DT)] for qc in range(NQB)]
            wvo_sb = persist.tile([P, CT, D], BF16, name="wvo_sb")
            bqk_sb = persist.tile([P, CT], F32, name="bqk_sb")
            bo_sb = persist.tile([P, D], F32, name="bo_sb")
            csum_sb = persist.tile([P, CT], F32, name="csum_sb")
            om_sb = persist.tile([P, P], BF16, name="om_sb")
            e0_sb = persist.tile([P, 2], BF16, name="e0_sb")
            ec_sb = persist.tile([P, 1], F32, name="ec_sb")
            sums_sb = persist.tile([P, QB], BF16, name="sums_sb")

            # ---- DMA plan ----
            # dma_start instructions round-robin the SW-DGE queues per
            # issue, and BLOCK the issuing engine when a ring is full --
            # so scalar (which runs the phase-1 psum evacs) must issue no
            # DMAs at all. sync+gpsimd carry everything in need order;
            # their streams are otherwise idle so ring backpressure is
            # harmless.
            nc.scalar.dma_start(bqk_sb, bqkh[:])  # needed at first evac
            # scalar's queue carries exactly the x0 pieces (first-needed;
            # its ring drains by ~20us, before the first evac must issue)
            # -- anything more on scalar's stream would block the evacs.
            crit = (nc.sync, nc.gpsimd)
            for it in range(DT):
                if it < 2:  # split the first M pieces for an earlier start
                    nc.sync.dma_start(m_tiles[it][:, 0:384],
                                      Mh[it * P:(it + 1) * P, 0:384])
                    nc.gpsimd.dma_start(m_tiles[it][:, 384:768],
                                        Mh[it * P:(it + 1) * P, 384:768])
                else:
                    crit[it % 2].dma_start(m_tiles[it],
                                           Mh[it * P:(it + 1) * P, :])
                if it == 0:
                    nc.scalar.dma_start(xt_tiles[0][0][:, 0:256],
                                        xT[0:P, 0:256])
                    nc.sync.dma_start(xt_tiles[0][0][:, 256:512],
                                      xT[0:P, 256:512])
                else:
                    nc.scalar.dma_start(xt_tiles[0][it],
                                        xT[it * P:(it + 1) * P, 0:QB])
            ci = 0
            for qc in range(1, NQB):
                for it in range(DT):
                    crit[ci % 2].dma_start(
                        xt_tiles[qc][it],
                        xT[it * P:(it + 1) * P, qc * QB:(qc + 1) * QB])
                    ci += 1
            bg = (nc.sync, nc.gpsimd)
            for t in range(CT):
                bg[t % 2].dma_start(ctx8_sb[:, t], ctx8T[t * P:(t + 1) * P, :])
            for kt_ in range(KT):
                bg[kt_ % 2].dma_start(ctxk_sb[:, kt_],
                                      ctxk8[kt_ * P:(kt_ + 1) * P, :])
            for t in range(CT):
                bg[t % 2].dma_start(wvo_sb[:, t], wvoh[t * P:(t + 1) * P, :])
            nc.sync.dma_start(csum_sb, csumb[:])
            nc.gpsimd.dma_start(bo_sb, bob[:])
            nc.sync.dma_start(om_sb, onesmat[:])
            nc.gpsimd.dma_start(e0_sb, e0two[:])
            nc.sync.dma_start(ec_sb, ecbh[:])

            # ===== phase 1: xm^T[c,q] = M^T @ x^T (+bqk), fp8 resident =====
            with tc.tile_pool(name="ps_xm", bufs=1, space="PSUM") as ps_xm:
                # chunk 0: it-outer across 6 live psum banks so the PE
                # starts as soon as the first (M[it], x[it]) pair lands
                pxms = [ps_xm.tile([P, QB], F32, name=f"pxm{cs}",
                                   tag=f"pxm{cs}") for cs in range(CT)]
                for it in range(DT):
                    for cs in range(CT):
                        nc.tensor.matmul(
                            pxms[cs], m_tiles[it][:, cs * P:(cs + 1) * P],
                            xt_tiles[0][it],
                            start=(it == 0), stop=(it == DT - 1))
                for cs in range(CT):
                    nc.scalar.activation(
                        xm_tiles[0][:, cs], pxms[cs],
                        AF.Identity, bias=bqk_sb[:, cs:cs + 1])
                # chunks 1..3: cs-outer (DMA is ahead; evac WAR long gone)
                for qc in range(1, NQB):
                    for cs in range(CT):
                        pxm = ps_xm.tile([P, QB], F32, name=f"pxm{cs}",
                                         tag=f"pxm{cs}")
                        for it in range(DT):
                            nc.tensor.matmul(
                                pxm, m_tiles[it][:, cs * P:(cs + 1) * P],
                                xt_tiles[qc][it],
                                start=(it == 0), stop=(it == DT - 1))
                        nc.scalar.activation(
                            xm_tiles[qc][:, cs], pxm,
                            AF.Identity, bias=bqk_sb[:, cs:cs + 1])

            # ================= phase 2: attention + fold-out ================
            with tc.tile_pool(name="p2_big", bufs=1) as p2_big, \
                 tc.tile_pool(name="p2_tmp", bufs=3) as p2_tmp, \
                 tc.tile_pool(name="p2_acc", bufs=16) as p2_acc, \
                 tc.tile_pool(name="p2_fin", bufs=4) as p2_fin, \
                 tc.tile_pool(name="ps_sc", bufs=2, space="PSUM") as ps_sc, \
                 tc.tile_pool(name="ps_po", bufs=2, space="PSUM") as ps_po, \
                 tc.tile_pool(name="ps_fin", bufs=2, space="PSUM") as ps_fin:
                # per-pair expt tiles: outp's kp-call then depends only on
                # its own pair's centering op, not all eight (dependency
                # tracking is per-tile)
                expt_tiles = [p2_big.tile([P, 2, QB], FP8, name=f"expt{kp}")
                              for kp in range(KT // 2)]
                outp_sb = p2_big.tile([P, CT, QB], BF16, name="outp_sb")
                for qb in range(NQB):
                    # ---- scores^T (fp8 DR) -> Exp bf16 -> centered fp8 ----
                    # scalar keeps only Exp (~1.1us/pair < PE 1.36us/pair);
                    # centering + pair-sums ride vector; the sum TREE runs
                    # after the last pair (its latency hides under outp)
                    pairs = []
                    for kp in range(KT // 2):
                        psc = ps_sc.tile([P, 2, QB], F32, name="psc", tag="psc")
                        for j in range(2):
                            kt_ = kp * 2 + j
                            for cs in range(0, CT, 2):
                                nc.tensor.matmul(
                                    psc[:, j],
                                    ctx8_sb[:, cs:cs + 2, kt_ * P:(kt_ + 1) * P],
                                    xm_tiles[qb][:, cs:cs + 2],
                                    start=(cs == 0), stop=(cs == CT - 2),
                                    perf_mode=DR)
                        tmp = p2_tmp.tile([P, 2, QB], BF16, name="tmp",
                                          tag="tmp")
                        nc.scalar.activation(tmp, psc, AF.Exp,
                                             scale=float(SCALE))
                        nc.vector.tensor_scalar(
                            expt_tiles[kp], tmp,
                            float(EC), float(ESC),
                            op0=ALU.subtract, op1=ALU.mult)
                        pair = p2_acc.tile([P, QB], BF16, name="pair",
                                           tag="acc")
                        nc.vector.tensor_add(pair, tmp[:, 0], tmp[:, 1])
                        pairs.append(pair)
                    while len(pairs) > 1:
                        nxt = []
                        for a, b in zip(pairs[0::2], pairs[1::2]):
                            nacc = p2_acc.tile([P, QB], BF16, name="acc",
                                               tag="acc")
                            nc.vector.tensor_add(nacc, a, b)
                            nxt.append(nacc)
                        pairs = nxt
                    acc = pairs[0]

                    # ---- outp^T[c,q] = ctxk8^T @ e8 over kt pairs (DR) ----
                    def outp_col(cc):
                        po = ps_po.tile([P, QB], F32, name="po", tag="po")
                        for kp in range(KT // 2):
                            nc.tensor.matmul(
                                po,
                                ctxk_sb[:, 2 * kp:2 * kp + 2,
                                        cc * P:(cc + 1) * P],
                                expt_tiles[kp],
                                start=(kp == 0), stop=(kp == KT // 2 - 1),
                                perf_mode=DR)
                        nc.scalar.activation(
                            outp_sb[:, cc], po, AF.Identity,
                            scale=1.0 / ESC, bias=csum_sb[:, cc:cc + 1])
                    # sums machinery is interleaved between outp cols so
                    # the PE never waits on the DVE acc chain or the
                    # scalar sums-copy: om after col2 (acc long done),
                    # transposes after col3 (copy done during col3).
                    outp_col(0)
                    outp_col(1)
                    outp_col(2)
                    psums = ps_fin.tile([P, QB], F32, name="psums", tag="pf")
                    nc.tensor.matmul(psums, om_sb, acc, start=True, stop=True)
                    nc.vector.tensor_copy(sums_sb, psums)
                    outp_col(3)
                    prt = ps_fin.tile([P, 8], F32, name="prt", tag="pf")
                    for qs in range(4):
                        nc.tensor.matmul(
                            prt[:, 2 * qs:2 * qs + 2],
                            sums_sb[:, qs * P:(qs + 1) * P], e0_sb,
                            start=True, stop=True)
                    recip = p2_acc.tile([P, 8], F32, name="recip",
                                        tag="recip")
                    nc.vector.reciprocal(recip, prt)
                    outp_col(4)
                    outp_col(5)
                    # ---- final = outp^T.T @ WVO; evac fuses *recip + bo'' ---
                    for qs in range(4):
                        for oc in range(2):
                            pf = ps_fin.tile([P, 512], F32, name="pf",
                                             tag="pf")
                            for cs in range(CT):
                                nc.tensor.matmul(
                                    pf, outp_sb[:, cs, qs * P:(qs + 1) * P],
                                    wvo_sb[:, cs, oc * 512:(oc + 1) * 512],
                                    start=(cs == 0), stop=(cs == CT - 1))
                            fin = p2_fin.tile([P, 512], BF16, name="fin",
                                              tag="fin")
                            nc.vector.scalar_tensor_tensor(
                                fin, pf, recip[:, 2 * qs:2 * qs + 1],
                                bo_sb[:, oc * 512:(oc + 1) * 512],
                                op0=ALU.mult, op1=ALU.add)
                            seng = (nc.sync, nc.gpsimd)[(qs * 2 + oc) % 2]
                            seng.dma_start(
                                out[qb * QB + qs * P: qb * QB + (qs + 1) * P,
                                    oc * 512:(oc + 1) * 512], fin)
    nc.finalize()
    _NC_CACHE["nc"] = nc
    return nc


def _host_prep(x, context, Wq, bq, Wk, bk, Wv, bv, Wo, bo):
    """Build the 8 per-core input maps (host-side weight folding)."""
    BF = ml_dtypes.bfloat16
    F8np = ml_dtypes.float8_e4m3
    x = np.asarray(x, dtype=np.float32)
    context = np.asarray(context, dtype=np.float32)
    Wq64 = np.asarray(Wq, np.float64)
    Wk64 = np.asarray(Wk, np.float64)
    Wv64 = np.asarray(Wv, np.float64)
    Wo64 = np.asarray(Wo, np.float64)
    M = Wq64.T @ Wk64                                 # [D, C]
    bqk = np.asarray(bq, np.float64) @ Wk64           # [C]
    WVO = (Wo64 @ Wv64).T                             # [C, D]
    bo_eff = np.asarray(bo, np.float64) + Wo64 @ np.asarray(bv, np.float64)

    Mh = np.ascontiguousarray(M.astype(np.float32)).astype(BF)
    wvoh = np.ascontiguousarray(WVO.astype(np.float32)).astype(BF)
    bqkh = np.ascontiguousarray(
        bqk.astype(np.float32).reshape(CT, P).T)      # [p, ct]
    bob = np.ascontiguousarray(
        np.broadcast_to(bo_eff.astype(np.float32)[None, :], (P, D)))
    onesmat = np.ones((P, P), np.float32).astype(BF)
    e0two = np.zeros((P, 2), np.float32)
    e0two[0, :] = 1.0
    e0two = e0two.astype(BF)
    ecbh = np.full((P, 1), -ESC * EC, np.float32)
    shared = dict(Mh=Mh, wvoh=wvoh, bqkh=bqkh, bob=bob,
                  onesmat=onesmat, e0two=e0two, ecbh=ecbh)
    xbf = x.astype(BF)
    in_maps = []
    for b in range(B):
        m = dict(shared)
        m["xT"] = np.ascontiguousarray(xbf[b].T)              # [D, SQ] bf16
        m["ctx8T"] = np.ascontiguousarray(context[b].T).astype(F8np)
        ck8 = np.ascontiguousarray(context[b]).astype(F8np)   # [SKV, C]
        m["ctxk8"] = ck8
        csum = EC * ck8.astype(np.float32).sum(axis=0)        # [C]
        m["csumb"] = np.ascontiguousarray(csum.reshape(CT, P).T)
        in_maps.append(m)
    return in_maps


def kernel(**inputs) -> np.ndarray:
    nc = build()
    in_maps = _host_prep(**inputs)
    res = run_bass_kernel_spmd(nc, in_maps, core_ids=list(range(B)))
    return np.stack(
        [res.results[b]["out"].astype(np.float32) for b in range(B)], axis=0)


# revision 17
# speedup vs baseline: 1.0126x; 1.0126x over previous
"""Trainium2 Bass kernel for nn_CrossAttention (B=8, Sq=Skv=2048, D=1024, C=768).

Strategy: data-parallel over batch — each of the 8 NeuronCores computes one
batch element's full cross-attention.

The projection chain is reassociated so every big contraction runs against
the NARROW context dim (C=768) instead of D=1024, and the K/V projections
disappear entirely (all exact identities, weights folded on host):

  scores = (x @ M + bqk) @ ctx^T          M   = Wq^T @ Wk   [D, C]
                                          bqk = bq @ Wk     [C]
  (bk drops: its score term is constant over k -> cancels in softmax)
  att    = softmax(scores / sqrt(D))
  final  = (e @ ctx)/sums @ WVO + bo''    WVO = (Wo @ Wv)^T [C, D]
                                          bo''= bo + Wo @ bv

Precision plan (2e-2 gate; this config emulates to ~1.67e-2):
  - scores matmul: fp8e4m3 DoubleRow (xm8 @ ctx8), 1/sqrt(D) folded into Exp.
  - att @ ctx matmul: fp8 DoubleRow with CENTERED attention weights:
      e_hat = EC + e8/ESC,  e8 = fp8((e - EC) * ESC)
    e clusters tightly around its mean (~1.1, std 0.52), so quantizing the
    centered residual cuts the fp8 noise ~3x. The rank-1 correction
    EC * colsum(ctxk8) is folded into the PSUM-evac bias (host-computed,
    exact, free). Softmax sums use the bf16 e (DVE pair-add tree), so the
    denominator matches the pre-quantization weights.
  - phase-1 projection and the final @WVO stay bf16: fp8 there busts the
    error budget (emulated 2.1e-2 / 2.5e-2).
  - output written bf16 (halves the tail DMA; +3e-4 error).

Per-core phases:
  phase 1 (q chunks of 512): xm^T[c,q] = M^T x^T + bqk -> fp8 resident.
    Chunk 0 runs it-OUTER (each arriving (M[it], x0[it]) DMA pair
    immediately feeds 6 matmuls across all cs psum banks) so the PE starts
    ~2us after the first tiles land instead of waiting for the full 2.5MB.
  phase 2 (per q block): scores^T fp8-DR -> Exp (bf16 e) -> centered fp8 e8;
    DVE accumulates softmax sums from bf16 e; ones-matmul + e0-trick
    transpose -> 1/sums. outp^T[c,q] = ctxk8^T @ e8 (fp8 DR over kt pairs),
    evac fuses *1/ESC + EC*colsum bias. final[q,o] = outp^T.T @ WVO (bf16);
    evac fuses (*recip + bo'') in one DVE op -> bf16 -> DMA out.

DMA: 3 queue-engines (sync/vector/gpsimd) carry the phase-1-critical M/x
stream in need order; scalar's queue carries ctx8/ctxk8/wvo/consts in the
background. Full-width per-it pieces (fewer, larger descriptors).
"""

import numpy as np
import ml_dtypes

import concourse.bass as bass  # noqa: F401
import concourse.mybir as mybir
import concourse.tile as tile
from concourse import bacc
from concourse.bass_utils import run_bass_kernel_spmd

# ---- problem shapes (hardcoded) ----
B, SQ, SKV, D, C = 8, 2048, 2048, 1024, 768
P = 128
DT = D // P          # 8  d-tiles
CT = C // P          # 6  c-tiles
KT = SKV // P        # 16 k-tiles
QB = 512             # q block width
NQB = SQ // QB       # 4 q blocks
SCALE = 1.0 / np.sqrt(np.float32(D))

EC = 1.105           # centering constant for e = exp(score)
ESC = 8.0            # fp8 scale for the centered residual

F32 = mybir.dt.float32
BF16 = mybir.dt.bfloat16
FP8 = mybir.dt.float8e4
AF = mybir.ActivationFunctionType
ALU = mybir.AluOpType
DR = mybir.MatmulPerfMode.DoubleRow

_NC_CACHE = {}


def build():
    if "nc" in _NC_CACHE:
        return _NC_CACHE["nc"]
    nc = bacc.Bacc(trn_type="TRN2", num_swdge_queues=4)

    # ---- DRAM I/O (per-core slices; names = in_map keys) ----
    xT = nc.dram_tensor("xT", [D, SQ], BF16, kind="ExternalInput")
    ctx8T = nc.dram_tensor("ctx8T", [C, SKV], FP8, kind="ExternalInput")
    ctxk8 = nc.dram_tensor("ctxk8", [SKV, C], FP8, kind="ExternalInput")
    Mh = nc.dram_tensor("Mh", [D, C], BF16, kind="ExternalInput")
    wvoh = nc.dram_tensor("wvoh", [C, D], BF16, kind="ExternalInput")
    bqkh = nc.dram_tensor("bqkh", [P, CT], F32, kind="ExternalInput")
    bob = nc.dram_tensor("bob", [P, D], F32, kind="ExternalInput")
    csumb = nc.dram_tensor("csumb", [P, CT], F32, kind="ExternalInput")
    onesmat = nc.dram_tensor("onesmat", [P, P], BF16, kind="ExternalInput")
    e0two = nc.dram_tensor("e0two", [P, 2], BF16, kind="ExternalInput")
    ecbh = nc.dram_tensor("ecbh", [P, 1], F32, kind="ExternalInput")
    out = nc.dram_tensor("out", [SQ, D], BF16, kind="ExternalOutput")

    with tile.TileContext(nc) as tc:
        with tc.tile_pool(name="persist", bufs=1) as persist:
            ctx8_sb = persist.tile([P, CT, SKV], FP8, name="ctx8_sb")
            ctxk_sb = persist.tile([P, KT, C], FP8, name="ctxk_sb")
            xm_tiles = [persist.tile([P, CT, QB], FP8,
                                     name=f"xm_sb{qc}")
                        for qc in range(NQB)]
            m_tiles = [persist.tile([P, C], BF16, name=f"m_sb{it}")
                       for it in range(DT)]
            xt_tiles = [[persist.tile([P, QB], BF16, name=f"xt{qc}_{it}")
                         for it in range(DT)] for qc in range(NQB)]
            wvo_sb = persist.tile([P, CT, D], BF16, name="wvo_sb")
            bqk_sb = persist.tile([P, CT], F32, name="bqk_sb")
            bo_sb = persist.tile([P, D], F32, name="bo_sb")
            csum_sb = persist.tile([P, CT], F32, name="csum_sb")
            om_sb = persist.tile([P, P], BF16, name="om_sb")
            e0_sb = persist.tile([P, 2], BF16, name="e0_sb")
            ec_sb = persist.tile([P, 1], F32, name="ec_sb")
            sums_sb = persist.tile([P, QB], BF16, name="sums_sb")

            # ---- DMA plan ----
            # phase-1 evacs ride VECTOR, so all three issue-capable
            # engines (sync/gpsimd/scalar) can carry a full round-robin
            # DMA share; ring backpressure never blocks compute (scalar's
            # first compute op is qb0's Exp at ~59us, long after its ring
            # drains). Pieces are issued in need order.
            engs = (nc.sync, nc.gpsimd, nc.scalar)
            ei = 0

            def issue(dst, src):
                nonlocal ei
                engs[ei % 3].dma_start(dst, src)
                ei += 1

            issue(bqk_sb, bqkh[:])
            for it in range(DT):
                if it == 0:  # split the first pair for an earlier start
                    issue(m_tiles[0][:, 0:384], Mh[0:P, 0:384])
                    issue(m_tiles[0][:, 384:768], Mh[0:P, 384:768])
                    issue(xt_tiles[0][0][:, 0:256], xT[0:P, 0:256])
                    issue(xt_tiles[0][0][:, 256:512], xT[0:P, 256:512])
                else:
                    issue(m_tiles[it], Mh[it * P:(it + 1) * P, :])
                    issue(xt_tiles[0][it], xT[it * P:(it + 1) * P, 0:QB])
            for qc in range(1, NQB):
                for it in range(DT):
                    issue(xt_tiles[qc][it],
                          xT[it * P:(it + 1) * P, qc * QB:(qc + 1) * QB])
            for t in range(CT):
                issue(ctx8_sb[:, t], ctx8T[t * P:(t + 1) * P, :])
            for kt_ in range(KT):
                issue(ctxk_sb[:, kt_], ctxk8[kt_ * P:(kt_ + 1) * P, :])
            for t in range(CT):
                issue(wvo_sb[:, t], wvoh[t * P:(t + 1) * P, :])
            issue(csum_sb, csumb[:])
            issue(bo_sb, bob[:])
            issue(om_sb, onesmat[:])
            issue(e0_sb, e0two[:])
            issue(ec_sb, ecbh[:])

            # ===== phase 1: xm^T[c,q] = M^T @ x^T (+bqk), fp8 resident =====
            with tc.tile_pool(name="ps_xm", bufs=1, space="PSUM") as ps_xm:
                # chunk 0: it-outer across 6 live psum banks so the PE
                # starts as soon as the first (M[it], x[it]) pair lands
                pxms = [ps_xm.tile([P, QB], F32, name=f"pxm{cs}",
                                   tag=f"pxm{cs}") for cs in range(CT)]
                for it in range(DT):
                    for cs in range(CT):
                        nc.tensor.matmul(
                            pxms[cs], m_tiles[it][:, cs * P:(cs + 1) * P],
                            xt_tiles[0][it],
                            start=(it == 0), stop=(it == DT - 1))
                for cs in range(CT):
                    nc.vector.tensor_scalar(
                        xm_tiles[0][:, cs], pxms[cs],
                        bqk_sb[:, cs:cs + 1], 1.0,
                        op0=ALU.add, op1=ALU.mult)
                # chunks 1..3: cs-outer (DMA is ahead; evac WAR long gone)
                for qc in range(1, NQB):
                    for cs in range(CT):
                        pxm = ps_xm.tile([P, QB], F32, name=f"pxm{cs}",
                                         tag=f"pxm{cs}")
                        for it in range(DT):
                            nc.tensor.matmul(
                                pxm, m_tiles[it][:, cs * P:(cs + 1) * P],
                                xt_tiles[qc][it],
                                start=(it == 0), stop=(it == DT - 1))
                        nc.vector.tensor_scalar(
                            xm_tiles[qc][:, cs], pxm,
                            bqk_sb[:, cs:cs + 1], 1.0,
                            op0=ALU.add, op1=ALU.mult)

            # ================= phase 2: attention + fold-out ================
            with tc.tile_pool(name="p2_big", bufs=1) as p2_big, \
                 tc.tile_pool(name="p2_tmp", bufs=4) as p2_tmp, \
                 tc.tile_pool(name="p2_acc", bufs=16) as p2_acc, \
                 tc.tile_pool(name="p2_fin", bufs=4) as p2_fin, \
                 tc.tile_pool(name="ps_sc", bufs=2, space="PSUM") as ps_sc, \
                 tc.tile_pool(name="ps_po", bufs=2, space="PSUM") as ps_po, \
                 tc.tile_pool(name="ps_fin", bufs=2, space="PSUM") as ps_fin:
                # per-pair expt tiles: outp's kp-call then depends only on
                # its own pair's centering op, not all eight (dependency
                # tracking is per-tile)
                expt_tiles = [p2_big.tile([P, 2, QB], FP8, name=f"expt{kp}")
                              for kp in range(KT // 2)]
                outp_sb = p2_big.tile([P, CT, QB], BF16, name="outp_sb")
                for qb in range(NQB):
                    # ---- scores^T (fp8 DR) -> Exp bf16 -> centered fp8 ----
                    # scalar keeps only Exp (~1.1us/pair < PE 1.36us/pair);
                    # centering + pair-sums ride vector; the sum TREE runs
                    # after the last pair (its latency hides under outp)
                    pairs = []
                    for kp in range(KT // 2):
                        psc = ps_sc.tile([P, 2, QB], F32, name="psc", tag="psc")
                        for j in range(2):
                            kt_ = kp * 2 + j
                            for cs in range(0, CT, 2):
                                nc.tensor.matmul(
                                    psc[:, j],
                                    ctx8_sb[:, cs:cs + 2, kt_ * P:(kt_ + 1) * P],
                                    xm_tiles[qb][:, cs:cs + 2],
                                    start=(cs == 0), stop=(cs == CT - 2),
                                    perf_mode=DR)
                        tmp = p2_tmp.tile([P, 2, QB], BF16, name="tmp",
                                          tag="tmp")
                        nc.scalar.activation(tmp, psc, AF.Exp,
                                             scale=float(SCALE))
                        nc.vector.tensor_scalar(
                            expt_tiles[kp], tmp,
                            float(EC), float(ESC),
                            op0=ALU.subtract, op1=ALU.mult)
                        pair = p2_acc.tile([P, QB], BF16, name="pair",
                                           tag="acc")
                        nc.vector.tensor_add(pair, tmp[:, 0], tmp[:, 1])
                        pairs.append(pair)
                    while len(pairs) > 1:
                        nxt = []
                        for a, b in zip(pairs[0::2], pairs[1::2]):
                            nacc = p2_acc.tile([P, QB], BF16, name="acc",
                                               tag="acc")
                            nc.vector.tensor_add(nacc, a, b)
                            nxt.append(nacc)
                        pairs = nxt
                    acc = pairs[0]

                    # ---- outp^T[c,q] = ctxk8^T @ e8 over kt pairs (DR) ----
                    def outp_col(cc):
                        po = ps_po.tile([P, QB], F32, name="po", tag="po")
                        for kp in range(KT // 2):
                            nc.tensor.matmul(
                                po,
                                ctxk_sb[:, 2 * kp:2 * kp + 2,
                                        cc * P:(cc + 1) * P],
                                expt_tiles[kp],
                                start=(kp == 0), stop=(kp == KT // 2 - 1),
                                perf_mode=DR)
                        nc.scalar.activation(
                            outp_sb[:, cc], po, AF.Identity,
                            scale=1.0 / ESC, bias=csum_sb[:, cc:cc + 1])
                    # sums machinery is interleaved between outp cols so
                    # the PE never waits on the DVE acc chain or the
                    # scalar sums-copy: om after col2 (acc long done),
                    # transposes after col3 (copy done during col3).
                    outp_col(0)
                    outp_col(1)
                    outp_col(2)
                    psums = ps_fin.tile([P, QB], F32, name="psums", tag="pf")
                    nc.tensor.matmul(psums, om_sb, acc, start=True, stop=True)
                    nc.vector.tensor_copy(sums_sb, psums)
                    outp_col(3)
                    prt = ps_fin.tile([P, 8], F32, name="prt", tag="pf")
                    for qs in range(4):
                        nc.tensor.matmul(
                            prt[:, 2 * qs:2 * qs + 2],
                            sums_sb[:, qs * P:(qs + 1) * P], e0_sb,
                            start=True, stop=True)
                    recip = p2_acc.tile([P, 8], F32, name="recip",
                                        tag="recip")
                    nc.vector.reciprocal(recip, prt)
                    outp_col(4)
                    outp_col(5)
                    # ---- final = outp^T.T @ WVO; evac fuses *recip + bo'' ---
                    for qs in range(4):
                        for oc in range(2):
                            pf = ps_fin.tile([P, 512], F32, name="pf",
                                             tag="pf")
                            for cs in range(CT):
                                nc.tensor.matmul(
                                    pf, outp_sb[:, cs, qs * P:(qs + 1) * P],
                                    wvo_sb[:, cs, oc * 512:(oc + 1) * 512],
                                    start=(cs == 0), stop=(cs == CT - 1))
                            fin = p2_fin.tile([P, 512], BF16, name="fin",
                                              tag="fin")
                            nc.vector.scalar_tensor_tensor(
                                fin, pf, recip[:, 2 * qs:2 * qs + 1],
                                bo_sb[:, oc * 512:(oc + 1) * 512],
                                op0=ALU.mult, op1=ALU.add)
                            seng = (nc.sync, nc.gpsimd)[(qs * 2 + oc) % 2]
                            seng.dma_start(
                                out[qb * QB + qs * P: qb * QB + (qs + 1) * P,
                                    oc * 512:(oc + 1) * 512], fin)
    nc.finalize()
    _NC_CACHE["nc"] = nc
    return nc


def _host_prep(x, context, Wq, bq, Wk, bk, Wv, bv, Wo, bo):
    """Build the 8 per-core input maps (host-side weight folding)."""
    BF = ml_dtypes.bfloat16
    F8np = ml_dtypes.float8_e4m3
    x = np.asarray(x, dtype=np.float32)
    context = np.asarray(context, dtype=np.float32)
    Wq64 = np.asarray(Wq, np.float64)
    Wk64 = np.asarray(Wk, np.float64)
    Wv64 = np.asarray(Wv, np.float64)
    Wo64 = np.asarray(Wo, np.float64)
    M = Wq64.T @ Wk64                                 # [D, C]
    bqk = np.asarray(bq, np.float64) @ Wk64           # [C]
    WVO = (Wo64 @ Wv64).T                             # [C, D]
    bo_eff = np.asarray(bo, np.float64) + Wo64 @ np.asarray(bv, np.float64)

    Mh = np.ascontiguousarray(M.astype(np.float32)).astype(BF)
    wvoh = np.ascontiguousarray(WVO.astype(np.float32)).astype(BF)
    bqkh = np.ascontiguousarray(
        bqk.astype(np.float32).reshape(CT, P).T)      # [p, ct]
    bob = np.ascontiguousarray(
        np.broadcast_to(bo_eff.astype(np.float32)[None, :], (P, D)))
    onesmat = np.ones((P, P), np.float32).astype(BF)
    e0two = np.zeros((P, 2), np.float32)
    e0two[0, :] = 1.0
    e0two = e0two.astype(BF)
    ecbh = np.full((P, 1), -ESC * EC, np.float32)
    shared = dict(Mh=Mh, wvoh=wvoh, bqkh=bqkh, bob=bob,
                  onesmat=onesmat, e0two=e0two, ecbh=ecbh)
    xbf = x.astype(BF)
    in_maps = []
    for b in range(B):
        m = dict(shared)
        m["xT"] = np.ascontiguousarray(xbf[b].T)              # [D, SQ] bf16
        m["ctx8T"] = np.ascontiguousarray(context[b].T).astype(F8np)
        ck8 = np.ascontiguousarray(context[b]).astype(F8np)   # [SKV, C]
        m["ctxk8"] = ck8
        csum = EC * ck8.astype(np.float32).sum(axis=0)        # [C]
        m["csumb"] = np.ascontiguousarray(csum.reshape(CT, P).T)
        in_maps.append(m)
    return in_maps


def kernel(**inputs) -> np.ndarray:
    nc = build()
    in_maps = _host_prep(**inputs)
    res = run_bass_kernel_spmd(nc, in_maps, core_ids=list(range(B)))
    return np.stack(
        [res.results[b]["out"].astype(np.float32) for b in range(B)], axis=0)
